# revision 19
# baseline (speedup 1.0000x reference)
"""PTv3 sparse encoder Trainium2 kernel.

Self-contained: kernel(pos, params) -> (coords [B,1024,3], feat [B,256,1024]).

Strategy: data-parallel over batch B=4 (one sample per NeuronCore, cores 0-3).
Host precomputes Morton-code sort permutations (composed gather index maps),
pooled-coords trajectory (== coords output), and folded weights (LayerNorm
gamma/beta folded into following linear layers, qkv bias augmentation, v-bias
folded into proj bias, attention scale folded into q weights).

Device computes the full feature network per sample:
  token-major activations x[128, T, C]; transposed companions built via
  PE-transpose for matmul inputs; patch attention with transposed scores
  (S^T[m,n]), ACT exp (no max subtraction: softmax is shift-invariant and
  scores are O(1) at this model scale), AV with V as stationary operand
  (col-tiled by head, ones-column producing softmax denominators), deferred
  normalization via SEL/BSEL matmuls + one tensor_tensor multiply.
  Token reorders between blocks via SWDGE dma_gather from a DRAM scratch.
Matmuls run in float32r (4x faster than fp32 on PE at N>=256).
"""
import numpy as np
from contextlib import ExitStack

B, N, GRID = 4, 8192, 0.003
CHS = [32, 64, 128, 256]
DEPTHS = [1, 1, 2, 2]
HEADS = [2, 4, 8, 16]
PATCH = 1024
HD = 16
CPAD = [64, 64, 128, 256]   # padded row size for 256B-multiple gather rows
NS = [8192, 4096, 2048, 1024]
F32 = np.float32

# ------------------------------------------------------------------
# host: morton / orders / coords
# ------------------------------------------------------------------

def _part1by2(x):
    x = x.astype(np.uint32) & np.uint32(0x3FF)
    x = (x | (x << np.uint32(16))) & np.uint32(0x030000FF)
    x = (x | (x << np.uint32(8))) & np.uint32(0x0300F00F)
    x = (x | (x << np.uint32(4))) & np.uint32(0x030C30C3)
    x = (x | (x << np.uint32(2))) & np.uint32(0x09249249)
    return x


def _morton(gc):
    return (_part1by2(gc[..., 0])
            | (_part1by2(gc[..., 1]) << np.uint32(1))
            | (_part1by2(gc[..., 2]) << np.uint32(2)))


def host_orders(pos_b):
    coords = pos_b.astype(F32)
    gmaps, pool_gmaps = [], []
    final_map = None
    for s in range(4):
        n = coords.shape[0]
        gs = GRID * (2 ** s)
        gc = np.clip(np.floor(coords / F32(gs)).astype(np.int32), 0, 1023)
        code_z = _morton(gc)
        code_zt = _morton(gc[..., ::-1])
        ordv = None
        stage_maps = []
        for bi in range(DEPTHS[s]):
            code = code_z if bi % 2 == 0 else code_zt
            perm = np.argsort(code, axis=0, kind="stable").astype(np.int64)
            if ordv is None:
                stage_maps.append(perm)
            else:
                stage_maps.append(np.argsort(ordv, kind="stable")[perm])
            ordv = perm
        gmaps.append(stage_maps)
        if s < 3:
            perm_z = np.argsort(code_z, axis=0, kind="stable").astype(np.int64)
            pm = np.argsort(ordv, kind="stable")[perm_z]
            if np.array_equal(pm, np.arange(n)):
                pm = None
            pool_gmaps.append(pm)
            cs = coords[perm_z]
            coords = cs.reshape(n // 2, 2, 3).mean(axis=1).astype(F32)
        else:
            final_map = np.argsort(ordv, kind="stable")
    return gmaps, pool_gmaps, final_map, coords


def wrap_idx(m):
    """index map -> int16 [128, n/16] wrapped (i -> [i%16, i//16], replicated)."""
    n = m.shape[0]
    t = np.zeros((16, n // 16), dtype=np.int16)
    t[np.arange(n) % 16, np.arange(n) // 16] = m.astype(np.int16)
    return np.tile(t, (8, 1))


# ------------------------------------------------------------------
# host: weight folding / packing
# ------------------------------------------------------------------

def _pack_rows(W):
    """[R, C] -> [128, ceil(R/128), C] zero padded."""
    R, C = W.shape
    nch = (R + 127) // 128
    out = np.zeros((128, nch, C), F32)
    for j in range(nch):
        r = W[j * 128:(j + 1) * 128]
        out[: r.shape[0], j] = r
    return out


def _gelu_tanh(x):
    c = F32(np.sqrt(2.0 / np.pi))
    x = x.astype(F32)
    return (F32(0.5) * x * (F32(1) + np.tanh(c * (x + F32(0.044715) * x ** 3)))).astype(F32)


def embed_row(params):
    e = params['embed']
    v = np.asarray(e['w'], F32)[0] + np.asarray(e['b'], F32)
    m = v.mean(dtype=F32)
    var = v.var(dtype=F32)
    r = F32(1.0) / np.sqrt(var + F32(1e-5))
    return _gelu_tanh((v - m) * r * np.asarray(e['g'], F32) + np.asarray(e['beta'], F32))


def fold_block(p, C, H):
    Hq = (H + 3) // 4
    w = {}
    Wc = np.asarray(p['cpe_w'], F32); bc = np.asarray(p['cpe_b'], F32)
    w['cpe'] = _pack_rows(np.concatenate([Wc, bc[None]], 0))

    g1 = np.asarray(p['ln1_g'], F32); b1 = np.asarray(p['ln1_b'], F32)
    Wqkv = np.asarray(p['qkv_w'], F32); cqkv = np.asarray(p['qkv_b'], F32)
    Wq, Wk, Wv = Wqkv[:, :C], Wqkv[:, C:2 * C], Wqkv[:, 2 * C:]
    cq, ck, cv = cqkv[:C], cqkv[C:2 * C], cqkv[2 * C:]
    sc = F32(0.25)
    Wq_f = np.concatenate([(g1[:, None] * Wq) * sc, ((b1 @ Wq + cq) * sc)[None]], 0)
    Wk_f = np.concatenate([g1[:, None] * Wk, (b1 @ Wk + ck)[None]], 0)
    Wv_f = np.concatenate([g1[:, None] * Wv, np.zeros((1, C), F32)], 0)
    bv_f = b1 @ Wv + cv
    qs = np.zeros((C + 1, Hq * 256), F32)
    for h in range(H):
        g, a = h // 4, h % 4
        qs[:, g * 256 + a * 32: g * 256 + a * 32 + 16] = Wq_f[:, h * 16:(h + 1) * 16]
        qs[:, g * 256 + 128 + a * 32: g * 256 + 128 + a * 32 + 16] = Wk_f[:, h * 16:(h + 1) * 16]
    w['qk'] = _pack_rows(qs)
    w['v'] = _pack_rows(Wv_f)

    Wp = np.asarray(p['proj_w'], F32)
    bp = (np.asarray(p['proj_b'], F32) + bv_f @ Wp).astype(F32)
    ps = np.zeros((Hq * 128, C), F32)
    for h in range(H):
        g, a = h // 4, h % 4
        ps[g * 128 + a * 32: g * 128 + a * 32 + 16] = Wp[h * 16:(h + 1) * 16]
    ps[16] = bp  # rides the s-row (==1 after normalize) of quad 0 head 0
    w['proj'] = ps.reshape(Hq, 128, C).transpose(1, 0, 2).copy()  # [128, Hq, C]

    g2 = np.asarray(p['ln2_g'], F32); b2 = np.asarray(p['ln2_b'], F32)
    W1 = np.asarray(p['fc1_w'], F32); c1 = np.asarray(p['fc1_b'], F32)
    w['fc1'] = _pack_rows(np.concatenate([g2[:, None] * W1, (b2 @ W1 + c1)[None]], 0))
    W2 = np.asarray(p['fc2_w'], F32); c2 = np.asarray(p['fc2_b'], F32)
    w['fc2'] = _pack_rows(np.concatenate([W2, c2[None]], 0))
    return w


def fold_pool(pp, C, Cn):
    W = np.asarray(pp['w'], F32); b = np.asarray(pp['b'], F32)
    return {
        'w': _pack_rows(np.concatenate([W, b[None]], 0)),
        'g': np.asarray(pp['g'], F32)[None, :],
        'beta': np.asarray(pp['beta'], F32)[None, :],
    }


def prep_params(params):
    t = {}
    for s in range(4):
        C, H = CHS[s], HEADS[s]
        for bi in range(DEPTHS[s]):
            w = fold_block(params['stages'][s]['blocks'][bi], C, H)
            for k, v in w.items():
                t[f"w{s}{bi}_{k}"] = np.ascontiguousarray(v)
        if s < 3:
            wp = fold_pool(params['stages'][s]['pool'], C, CHS[s + 1])
            t[f"p{s}_w"] = np.ascontiguousarray(wp['w'])
            t[f"p{s}_g"] = wp['g']
            t[f"p{s}_beta"] = wp['beta']
    x0 = np.zeros((128, 64), F32)
    x0[:, :32] = embed_row(params)[None, :]
    t["x0bc"] = x0
    sel = np.zeros((128, 4), F32)
    for a in range(4):
        sel[32 * a + 16, a] = 1.0
    t["selc"] = sel
    bsel = np.zeros((4, 128), F32)
    for a in range(4):
        bsel[a, 32 * a:32 * a + 32] = 1.0
    t["bselc"] = bsel
    return t


def prep_sample(pos_b):
    gmaps, pool_gmaps, final_map, coords = host_orders(pos_b)
    t = {}
    t["gi_s1"] = wrap_idx(gmaps[1][0])
    t["gi_s2"] = wrap_idx(gmaps[2][0])
    t["gi_s2b"] = wrap_idx(gmaps[2][1])
    t["gi_s2p"] = wrap_idx(pool_gmaps[2])
    t["gi_s3"] = wrap_idx(gmaps[3][0])
    t["gi_s3b"] = wrap_idx(gmaps[3][1])
    t["gi_fin"] = wrap_idx(final_map)
    assert pool_gmaps[0] is None and pool_gmaps[1] is None
    return t, coords


# ------------------------------------------------------------------
# bass kernel
# ------------------------------------------------------------------

_CACHED = {}

WSHAPES = {}
for _s in range(4):
    _C, _H = CHS[_s], HEADS[_s]
    _Hq = (_H + 3) // 4
    _n1 = (_C + 1 + 127) // 128
    WSHAPES[_s] = {
        "cpe": (128, _n1, _C),
        "qk": (128, _n1, _Hq * 256),
        "v": (128, _n1, _C),
        "proj": (128, _Hq, _C),
        "fc1": (128, _n1, 2 * _C),
        "fc2": (128, (2 * _C + 1 + 127) // 128, _C),
    }


def build_kernel():
    import concourse.bacc as bacc
    import concourse.bass as bass
    import concourse.tile as tile
    from concourse import mybir
    from concourse.masks import make_identity

    f32 = mybir.dt.float32
    f32r = mybir.dt.float32r
    i16 = mybir.dt.int16
    AF = mybir.ActivationFunctionType
    OP = mybir.AluOpType

    nc = bacc.Bacc("TRN2", target_bir_lowering=False, debug=False)

    din = {}

    def dI(name, shape, dt=f32):
        din[name] = nc.dram_tensor(name, list(shape), dt, kind="ExternalInput")
        return din[name]

    for s in range(4):
        C = CHS[s]
        for bi in range(DEPTHS[s]):
            for k, shp in WSHAPES[s].items():
                dI(f"w{s}{bi}_{k}", shp)
        if s < 3:
            dI(f"p{s}_w", (128, (C + 1 + 127) // 128, 2 * C))
            dI(f"p{s}_g", (1, 2 * C))
            dI(f"p{s}_beta", (1, 2 * C))
    dI("x0bc", (128, 64))
    dI("selc", (128, 4))
    dI("bselc", (4, 128))
    for nm, n in [("gi_s1", 4096), ("gi_s2", 2048), ("gi_s2b", 2048),
                  ("gi_s2p", 2048), ("gi_s3", 1024), ("gi_s3b", 1024),
                  ("gi_fin", 1024)]:
        dI(nm, (128, n // 16), i16)

    d_out = nc.dram_tensor("feat_out", [1024, 256], f32, kind="ExternalOutput")
    d_dbg = nc.dram_tensor("dbg_out", [128, 4096], f32, kind="ExternalOutput")
    d_scr = nc.dram_tensor("scratch", [8192, 64], f32, kind="Internal")
    d_scr2 = nc.dram_tensor("scratch2", [2048, 128], f32, kind="Internal")
    d_scr3 = nc.dram_tensor("scratch3", [1024, 256], f32, kind="Internal")

    with tile.TileContext(nc) as tc, ExitStack() as ctx:
        ctx.enter_context(nc.allow_low_precision(
            reason="fp32r rounding of matmul inputs (intentional)"))
        sb1 = ctx.enter_context(tc.tile_pool(name="persist", bufs=1))
        wpool = ctx.enter_context(tc.tile_pool(name="wpool", bufs=1))
        spool = ctx.enter_context(tc.tile_pool(name="trans", bufs=2))
        hpool = ctx.enter_context(tc.tile_pool(name="htrans", bufs=1))
        pp_big = ctx.enter_context(tc.tile_pool(name="ppb", bufs=3, space="PSUM"))
        pp_o = ctx.enter_context(tc.tile_pool(name="ppo", bufs=1, space="PSUM"))

        x_flat = sb1.tile([128, 4096], f32, tag="x")
        q_sp = sb1.tile([128, 4, 1024], f32r, tag="qsp")
        k_sp = sb1.tile([128, 4, 1024], f32r, tag="ksp")
        v_buf = sb1.tile([128, 4096], f32, tag="vbuf")
        xTa = sb1.tile([128, 2, 1024], f32r, tag="xTa")
        hTa = sb1.tile([128, 2, 1024], f32r, tag="hTa")
        on_flat = sb1.tile([128, 4096], f32r, tag="on")
        ident = sb1.tile([128, 128], f32, tag="ident")
        ident_r = sb1.tile([128, 128], f32r, tag="identr")
        ones1 = sb1.tile([1, 1024], f32r, tag="ones1")
        eps_t = sb1.tile([128, 1], f32, tag="eps")
        sel_t = sb1.tile([128, 4], f32r, tag="sel")
        bsel_t = sb1.tile([4, 128], f32r, tag="bsel")

        make_identity(nc, ident[:])
        nc.vector.tensor_copy(ident_r[:], ident[:])
        nc.vector.memset(ones1[:].bitcast(f32), 1.0)
        nc.vector.memset(eps_t[:], 1e-5)
        nc.gpsimd.dma_start(sel_t[:], din["selc"][:].bitcast(f32r))
        nc.gpsimd.dma_start(bsel_t[:], din["bselc"][:].bitcast(f32r))

        # pool-stage small weights persistent
        wsb = {}
        for s in range(3):
            C = CHS[s]
            d = din[f"p{s}_w"]
            t = sb1.tile(list(d.shape), f32r, tag=f"p{s}w")
            nc.sync.dma_start(t[:], d[:].bitcast(f32r))
            wsb[f"p{s}_w"] = t
            for nm in ["g", "beta"]:
                dd = din[f"p{s}_{nm}"]
                tt = sb1.tile([128, 2 * C], f32, tag=f"p{s}{nm}")
                src = bass.AP(tensor=dd[:].tensor, offset=dd[:].offset,
                              ap=[[0, 128]] + [list(a) for a in dd[:].ap[1:]])
                nc.gpsimd.dma_start(tt[:], src)
                wsb[f"p{s}_{nm}"] = tt
        gidx = {}
        for nm in ["gi_s1", "gi_s2", "gi_s2b", "gi_s2p", "gi_s3", "gi_s3b", "gi_fin"]:
            t = sb1.tile(list(din[nm].shape), i16, tag=nm)
            nc.sync.dma_start(t[:], din[nm][:])
            gidx[nm] = t

        gsem = nc.alloc_semaphore("gsem")
        sem_count = [0]

        def x_view(s):
            T = NS[s] // 128
            return x_flat[:, 0:T * CPAD[s]].rearrange("p (t c) -> p t c", c=CPAD[s])

        def dram_gather(s_from, s_to, idx_t, n_to):
            Cp = CPAD[s_from]
            scr = {64: d_scr, 128: d_scr2, 256: d_scr3}[Cp]
            xv = x_view(s_from)
            T_from = NS[s_from] // 128
            dview = scr[:].rearrange("(t p) c -> p t c", p=128)
            xo = x_view(s_to)
            with tc.tile_critical():
                sem_count[0] += 16
                nc.gpsimd.dma_start(
                    dview[:, 0:T_from, 0:Cp], xv[:, :, :]).then_inc(gsem, 16)
                nc.gpsimd.wait_ge(gsem, sem_count[0])
                sem_count[0] += 16
                nc.gpsimd.dma_gather(
                    out_ap=xo[:, 0:n_to // 128, :],
                    in_ap=scr[0:NS[s_from], :],
                    idxs_ap=idx_t[:],
                    num_idxs=n_to,
                    num_idxs_reg=n_to,
                    elem_size=Cp,
                    single_packet=False,
                ).then_inc(gsem, 16)
                nc.gpsimd.wait_ge(gsem, sem_count[0])

        def transpose_in(src_ap_fn, ncols, dst_chunk, n_tok):
            """PE-transpose n_tok//128 token-major tiles into dst rows."""
            ntile = n_tok // 128
            for grp in range(0, ntile, 4):
                gn = min(4, ntile - grp)
                pt = pp_big.tile([128, 4, 128], f32, tag="big")
                for t in range(gn):
                    nc.tensor.transpose(
                        pt[0:ncols, t, :], src_ap_fn(grp + t), ident[:])
                nc.vector.tensor_copy(
                    dst_chunk[0:ncols, grp * 128:(grp + gn) * 128],
                    pt[0:ncols, 0:gn, :])

        def load_w(s, bi):
            lw = {}
            for k, shp in WSHAPES[s].items():
                t = wpool.tile(list(shp), f32r, tag="w_" + k)
                nc.sync.dma_start(t[:], din[f"w{s}{bi}_{k}"][:].bitcast(f32r))
                lw[k] = t
            return lw

        def ln_apply(xv_ap_fn, out_t, C, gn):
            """LayerNorm (no affine) of gn token tiles -> out_t[:, t, 0:C]."""
            stats = spool.tile([128, 8, 6], f32, tag="st")
            mv = spool.tile([128, 8, 2], f32, tag="mv")
            rs = spool.tile([128, 8], f32, tag="rs")
            for t in range(gn):
                nc.vector.bn_stats(stats[:, t, :], xv_ap_fn(t))
                nc.vector.bn_aggr(mv[:, t, :], stats[:, t, :])
            nc.scalar.activation(rs[:, 0:gn], mv[:, 0:gn, 1], AF.Sqrt,
                                 bias=eps_t[:])
            nc.vector.reciprocal(rs[:, 0:gn], rs[:, 0:gn])
            for t in range(gn):
                rbc = bass.AP(tensor=rs[:].tensor, offset=rs[:].offset + t,
                              ap=[list(rs[:].ap[0]), [0, C]])
                nc.vector.scalar_tensor_tensor(
                    out=out_t[:, t, 0:C], in0=xv_ap_fn(t),
                    scalar=mv[:, t, 0:1], in1=rbc,
                    op0=OP.subtract, op1=OP.mult)

        # ================= block =================
        def run_block(s, bi):
            C, H = CHS[s], HEADS[s]
            n = NS[s]
            P = n // PATCH
            Hq = (H + 3) // 4
            F = 2 * C
            xv = x_view(s)
            lw = load_w(s, bi)
            inline_ones = C < 128
            nch = 1 if inline_ones else C // 128
            kkx = C + 1 if inline_ones else 128
            jr, rr = C // 128, C % 128          # bias row position (rr==0 unless inline)
            fc_inline = (F + 1) <= 128
            nF = (F + 127) // 128
            fjr, frr = F // 128, F % 128
            vv = v_buf[:, 0:8 * H * 32].rearrange("p (t h e) -> p t h e", t=8, h=H)
            o_n = on_flat[:, :].rearrange("p (g c) -> p g c", c=1024)
            gT = o_n

            for p in range(P):
                pt0 = p * 8

                if inline_ones:
                    nc.vector.memset(xTa[C:C + 1, 0, 0:1024].bitcast(f32), 1.0)
                for j in range(nch):
                    cw = min(128, C - j * 128)
                    transpose_in(
                        lambda t, j=j, cw=cw: xv[:, pt0 + t, j * 128:j * 128 + cw],
                        cw, xTa[:, j, :], PATCH)

                # cpe + residual
                for gt in range(0, 8, 4):
                    pc = pp_big.tile([128, 4, 256], f32, tag="big")
                    for t in range(4):
                        tok = (gt + t) * 128
                        for j in range(nch):
                            nc.tensor.matmul(
                                pc[:, t, 0:C], xTa[0:kkx, j, tok:tok + 128],
                                lw["cpe"][0:kkx, j, 0:C],
                                start=(j == 0), stop=(j == nch - 1 and inline_ones))
                        if not inline_ones:
                            nc.tensor.matmul(
                                pc[:, t, 0:C], ones1[:, 0:128],
                                lw["cpe"][rr:rr + 1, jr, 0:C],
                                start=False, stop=True)
                    for t in range(4):
                        nc.vector.tensor_tensor(
                            out=xv[:, pt0 + gt + t, 0:C], in0=pc[:, t, 0:C],
                            in1=xv[:, pt0 + gt + t, 0:C], op=OP.add)

                # ln1 -> h -> hT
                h_t = hpool.tile([128, 8, 256], f32, tag="h")
                ln_apply(lambda t: xv[:, pt0 + t, 0:C], h_t, C, 8)
                if inline_ones:
                    nc.vector.memset(hTa[C:C + 1, 0, 0:1024].bitcast(f32), 1.0)
                for j in range(nch):
                    cw = min(128, C - j * 128)
                    transpose_in(
                        lambda t, j=j, cw=cw: h_t[:, t, j * 128:j * 128 + cw],
                        cw, hTa[:, j, :], PATCH)

                # qkT spread
                for g in range(Hq):
                    for jk, dst in ((0, q_sp), (1, k_sp)):
                        pq = pp_big.tile([128, 1024], f32, tag="big")
                        colb = g * 256 + jk * 128
                        for nh in range(2):
                            for j in range(nch):
                                nc.tensor.matmul(
                                    pq[:, nh * 512:(nh + 1) * 512],
                                    lw["qk"][0:kkx, j, colb:colb + 128],
                                    hTa[0:kkx, j, nh * 512:(nh + 1) * 512],
                                    start=(j == 0),
                                    stop=(j == nch - 1 and inline_ones))
                            if not inline_ones:
                                nc.tensor.matmul(
                                    pq[:, nh * 512:(nh + 1) * 512],
                                    lw["qk"][rr:rr + 1, jr, colb:colb + 128],
                                    ones1[:, nh * 512:(nh + 1) * 512],
                                    start=False, stop=True)
                        nc.vector.tensor_copy(dst[:, g, :], pq[:])

                # V token-major (no bias)
                for gt in range(0, 8, 4):
                    pv = pp_big.tile([128, 4, 256], f32, tag="big")
                    for t in range(4):
                        tok = (gt + t) * 128
                        for j in range(nch):
                            kk = C if inline_ones else 128
                            nc.tensor.matmul(
                                pv[:, t, 0:H * 16],
                                hTa[0:kk, j, tok:tok + 128],
                                lw["v"][0:kk, j, 0:H * 16],
                                start=(j == 0), stop=(j == nch - 1))
                    src = pv[:, 0:4, 0:H * 16].rearrange("p t (h e) -> p t h e", e=16)
                    nc.vector.tensor_copy(vv[:, gt:gt + 4, :, 0:16], src)

                # attention
                for g in range(Hq):
                    hg = min(4, H - 4 * g)
                    R = hg * 32
                    po = pp_o.tile([128, 1024], f32, tag="opsum")
                    for nh in range(2):
                        for mt in range(8):
                            for d2 in range(0, hg, 2):
                                dn = min(2, hg - d2)
                                psc = pp_big.tile([128, 2, 512], f32, tag="big")
                                for a2 in range(dn):
                                    a = d2 + a2
                                    nc.tensor.matmul(
                                        psc[:, a2, :],
                                        k_sp[32 * a:32 * a + 16, g,
                                             mt * 128:(mt + 1) * 128],
                                        q_sp[32 * a:32 * a + 16, g,
                                             nh * 512:(nh + 1) * 512],
                                        start=True, stop=True,
                                        tile_position=(32 * a, 0))
                                E = spool.tile([128, 2, 512], f32, tag="E")
                                nc.scalar.activation(
                                    E[:, 0:dn, :], psc[:, 0:dn, :], AF.Exp)
                                for a2 in range(dn):
                                    a = d2 + a2
                                    nc.tensor.matmul(
                                        po[32 * a:32 * a + 32,
                                           nh * 512:(nh + 1) * 512],
                                        vv[:, mt, 4 * g + a, 0:32],
                                        E[:, a2, :],
                                        start=(mt == 0), stop=(mt == 7),
                                        tile_position=(0, 32 * a),
                                        skip_group_check=True)
                    o_s = spool.tile([128, 1024], f32r, tag="os")
                    nc.vector.tensor_copy(o_s[0:R, :], po[0:R, :])
                    ps4 = pp_big.tile([128, 1024], f32, tag="big")
                    for nh in range(2):
                        nc.tensor.matmul(
                            ps4[0:hg, nh * 512:(nh + 1) * 512],
                            sel_t[0:R, 0:hg],
                            o_s[0:R, nh * 512:(nh + 1) * 512],
                            start=True, stop=True)
                    invs = spool.tile([4, 1024], f32r, tag="invs")
                    nc.vector.reciprocal(invs[0:hg, :], ps4[0:hg, 0:1024])
                    pB = pp_big.tile([128, 1024], f32, tag="big")
                    for nh in range(2):
                        nc.tensor.matmul(
                            pB[0:R, nh * 512:(nh + 1) * 512],
                            bsel_t[0:hg, 0:R],
                            invs[0:hg, nh * 512:(nh + 1) * 512],
                            start=True, stop=True)
                    nc.vector.tensor_tensor(
                        out=o_n[0:R, g, :], in0=o_s[0:R, :], in1=pB[0:R, :],
                        op=OP.mult)

                # proj + residual
                for gt in range(0, 8, 4):
                    pc = pp_big.tile([128, 4, 256], f32, tag="big")
                    for t in range(4):
                        tok = (gt + t) * 128
                        for g in range(Hq):
                            R = min(4, H - 4 * g) * 32
                            nc.tensor.matmul(
                                pc[:, t, 0:C],
                                o_n[0:R, g, tok:tok + 128],
                                lw["proj"][0:R, g, 0:C],
                                start=(g == 0), stop=(g == Hq - 1))
                    for t in range(4):
                        nc.vector.tensor_tensor(
                            out=xv[:, pt0 + gt + t, 0:C], in0=pc[:, t, 0:C],
                            in1=xv[:, pt0 + gt + t, 0:C], op=OP.add)

                # ln2 -> h -> hT
                h2 = hpool.tile([128, 8, 256], f32, tag="h")
                ln_apply(lambda t: xv[:, pt0 + t, 0:C], h2, C, 8)
                if inline_ones:
                    nc.vector.memset(hTa[C:C + 1, 0, 0:1024].bitcast(f32), 1.0)
                for j in range(nch):
                    cw = min(128, C - j * 128)
                    transpose_in(
                        lambda t, j=j, cw=cw: h2[:, t, j * 128:j * 128 + cw],
                        cw, hTa[:, j, :], PATCH)

                # fc1 -> gelu (gT = on_flat chunks)
                for mj in range(nF):
                    fm = min(128, F - mj * 128)
                    pf = pp_big.tile([128, 1024], f32, tag="big")
                    for nh in range(2):
                        for j in range(nch):
                            nc.tensor.matmul(
                                pf[0:fm, nh * 512:(nh + 1) * 512],
                                lw["fc1"][0:kkx, j, mj * 128:mj * 128 + fm],
                                hTa[0:kkx, j, nh * 512:(nh + 1) * 512],
                                start=(j == 0), stop=(j == nch - 1 and inline_ones))
                        if not inline_ones:
                            nc.tensor.matmul(
                                pf[0:fm, nh * 512:(nh + 1) * 512],
                                lw["fc1"][rr:rr + 1, jr, mj * 128:mj * 128 + fm],
                                ones1[:, nh * 512:(nh + 1) * 512],
                                start=False, stop=True)
                    nc.scalar.activation(gT[0:fm, mj, :], pf[0:fm, :],
                                         AF.Gelu_apprx_tanh)
                if fc_inline:
                    nc.vector.memset(gT[F:F + 1, 0, 0:1024].bitcast(f32), 1.0)

                # fc2 + residual
                for gt in range(0, 8, 4):
                    pc2 = pp_big.tile([128, 4, 256], f32, tag="big")
                    for t in range(4):
                        tok = (gt + t) * 128
                        for mj in range(nF):
                            kk = F + 1 if fc_inline else 128
                            nc.tensor.matmul(
                                pc2[:, t, 0:C],
                                gT[0:kk, mj, tok:tok + 128],
                                lw["fc2"][0:kk, mj, 0:C],
                                start=(mj == 0), stop=(mj == nF - 1 and fc_inline))
                        if not fc_inline:
                            nc.tensor.matmul(
                                pc2[:, t, 0:C], ones1[:, 0:128],
                                lw["fc2"][frr:frr + 1, fjr, 0:C],
                                start=False, stop=True)
                    for t in range(4):
                        nc.vector.tensor_tensor(
                            out=xv[:, pt0 + gt + t, 0:C], in0=pc2[:, t, 0:C],
                            in1=xv[:, pt0 + gt + t, 0:C], op=OP.add)

        # ================= pool =================
        def run_pool(s):
            C = CHS[s]
            Cn = 2 * C
            n = NS[s]
            n2 = n // 2
            xv = x_view(s)
            inline_ones = C < 128
            nch = 1 if inline_ones else C // 128
            kkx = C + 1 if inline_ones else 128
            jr, rr = C // 128, C % 128
            nM = (Cn + 127) // 128
            ymT = on_flat[:, 0:nM * n2].rearrange("p (m c) -> p m c", m=nM)
            for pc in range(n // 1024):
                if inline_ones:
                    nc.vector.memset(xTa[C:C + 1, 0, 0:1024].bitcast(f32), 1.0)
                for j in range(nch):
                    cw = min(128, C - j * 128)
                    transpose_in(
                        lambda t, j=j, cw=cw: xv[:, pc * 8 + t, j * 128:j * 128 + cw],
                        cw, xTa[:, j, :], PATCH)
                for mj in range(nM):
                    fm = min(128, Cn - mj * 128)
                    py = pp_big.tile([128, 1024], f32, tag="big")
                    for nh in range(2):
                        for j in range(nch):
                            nc.tensor.matmul(
                                py[0:fm, nh * 512:(nh + 1) * 512],
                                wsb[f"p{s}_w"][0:kkx, j, mj * 128:mj * 128 + fm],
                                xTa[0:kkx, j, nh * 512:(nh + 1) * 512],
                                start=(j == 0), stop=(j == nch - 1 and inline_ones))
                        if not inline_ones:
                            nc.tensor.matmul(
                                py[0:fm, nh * 512:(nh + 1) * 512],
                                wsb[f"p{s}_w"][rr:rr + 1, jr, mj * 128:mj * 128 + fm],
                                ones1[:, nh * 512:(nh + 1) * 512],
                                start=False, stop=True)
                    ptmp = spool.tile([128, 512], f32, tag="pm")
                    nc.vector.tensor_copy(ptmp[0:fm, :], py[0:fm, 0:1024:2])
                    nc.vector.tensor_tensor(
                        out=ymT[0:fm, mj, pc * 512:(pc + 1) * 512],
                        in0=ptmp[0:fm, :],
                        in1=py[0:fm, 1:1024:2], op=OP.max)
            # transpose back token-major into next-stage x view
            xo = x_view(s + 1)
            T2 = n2 // 128
            for grp in range(0, T2, 4):
                gn = min(4, T2 - grp)
                for mj in range(nM):
                    ncols = min(128, Cn - mj * 128)
                    ptb = pp_big.tile([128, 4, 128], f32r, tag="big")
                    for t in range(gn):
                        nc.tensor.transpose(
                            ptb[0:128, t, 0:ncols],
                            ymT[0:ncols, mj, (grp + t) * 128:(grp + t + 1) * 128],
                            ident_r[0:ncols, 0:ncols])
                    nc.vector.tensor_copy(
                        xo[:, grp:grp + gn, mj * 128:mj * 128 + ncols],
                        ptb[:, 0:gn, 0:ncols])
            # ln * g + beta, gelu
            Gt = wsb[f"p{s}_g"]
            Bt = wsb[f"p{s}_beta"]
            for grp in range(0, T2, 8):
                gn = min(8, T2 - grp)
                h_t = hpool.tile([128, 8, 256], f32, tag="h")
                ln_apply(lambda t: xo[:, grp + t, 0:Cn], h_t, Cn, gn)
                gbc = bass.AP(tensor=Gt[:].tensor, offset=Gt[:].offset,
                              ap=[list(Gt[:].ap[0]), [0, gn], [1, Cn]])
                bbc = bass.AP(tensor=Bt[:].tensor, offset=Bt[:].offset,
                              ap=[list(Bt[:].ap[0]), [0, gn], [1, Cn]])
                nc.vector.tensor_tensor(out=h_t[:, 0:gn, 0:Cn],
                                        in0=h_t[:, 0:gn, 0:Cn], in1=gbc, op=OP.mult)
                nc.vector.tensor_tensor(out=h_t[:, 0:gn, 0:Cn],
                                        in0=h_t[:, 0:gn, 0:Cn], in1=bbc, op=OP.add)
                nc.scalar.activation(xo[:, grp:grp + gn, 0:Cn],
                                     h_t[:, 0:gn, 0:Cn], AF.Gelu_apprx_tanh)

        def stage_init(s):
            H = HEADS[s]
            vv = v_buf[:, 0:8 * H * 32].rearrange("p (t h e) -> p t h e", t=8, h=H)
            nc.vector.memset(vv[:, :, :, 16:17], 1.0)
            nc.vector.memset(vv[:, :, :, 17:32], 0.0)

        # ================= main =================
        x0v = x_view(0)
        src = bass.AP(tensor=din["x0bc"][:].tensor, offset=din["x0bc"][:].offset,
                      ap=[[64, 128], [0, 64], [1, 64]])
        nc.gpsimd.dma_start(x0v[:, :, :], src)

        import os as _os
        _nph = int(_os.environ.get("KBENCH_PHASES", "99"))
        phases = [
            lambda: (stage_init(0), run_block(0, 0)),
            lambda: run_pool(0),
            lambda: dram_gather(1, 1, gidx["gi_s1"], 4096),
            lambda: (stage_init(1), run_block(1, 0)),
            lambda: run_pool(1),
            lambda: dram_gather(2, 2, gidx["gi_s2"], 2048),
            lambda: (stage_init(2), run_block(2, 0)),
            lambda: dram_gather(2, 2, gidx["gi_s2b"], 2048),
            lambda: run_block(2, 1),
            lambda: dram_gather(2, 2, gidx["gi_s2p"], 2048),
            lambda: run_pool(2),
            lambda: dram_gather(3, 3, gidx["gi_s3"], 1024),
            lambda: (stage_init(3), run_block(3, 0)),
            lambda: dram_gather(3, 3, gidx["gi_s3b"], 1024),
            lambda: run_block(3, 1),
            lambda: dram_gather(3, 3, gidx["gi_fin"], 1024),
        ]
        for _f in phases[:_nph]:
            _f()
        nc.sync.dma_start(d_dbg[:], x_flat[:, :])
        if _nph >= len(phases):
            xf = x_view(3)
            nc.sync.dma_start(
                d_out[:].rearrange("(t p) c -> p t c", p=128), xf[:, :, :])
        else:
            nc.vector.memset(x_flat[0:1, 0:1], 0.0)
            nc.sync.dma_start(
                d_out[:].rearrange("(t p) c -> p t c", p=128),
                x_flat[:, 0:2048].rearrange("p (t c) -> p t c", c=256))

    nc.finalize()
    return nc


def _make_runner(nc, n_cores):
    """Cached PJRT runner (mirrors bass2jax.run_bass_via_pjrt but reusable)."""
    import jax
    import numpy as _np
    from jax.sharding import Mesh, PartitionSpec
    from jax.experimental.shard_map import shard_map
    import concourse.mybir as mybir
    from concourse.bass2jax import install_neuronx_cc_hook, _bass_exec_p

    install_neuronx_cc_hook()
    in_names, out_names, out_avals = [], [], []
    for alloc in nc.m.functions[0].allocations:
        if not isinstance(alloc, mybir.MemoryLocationSet):
            continue
        name = alloc.memorylocations[0].name
        if alloc.kind == "ExternalInput":
            in_names.append(name)
        elif alloc.kind == "ExternalOutput":
            shape = tuple(alloc.tensor_shape)
            dtype = mybir.dt.np(alloc.dtype)
            out_names.append(name)
            out_avals.append(jax.core.ShapedArray(shape, dtype))
    n_params = len(in_names)
    n_outs = len(out_avals)
    all_names = in_names + out_names

    def _body(*args):
        outs = _bass_exec_p.bind(
            *args,
            out_avals=tuple(out_avals),
            in_names=tuple(all_names),
            out_names=tuple(out_names),
            lowering_input_output_aliases=(),
            sim_require_finite=True,
            sim_require_nnan=True,
            nc=nc,
        )
        return tuple(outs)

    devices = jax.devices()[:n_cores]
    mesh = Mesh(_np.asarray(devices), ("core",))
    in_specs = (PartitionSpec("core"),) * (n_params + n_outs)
    out_specs = (PartitionSpec("core"),) * n_outs
    donate = tuple(range(n_params, n_params + n_outs))
    sharded = jax.jit(
        shard_map(_body, mesh=mesh, in_specs=in_specs, out_specs=out_specs,
                  check_rep=False),
        donate_argnums=donate, keep_unused=True)

    pid_name = nc.partition_id_tensor.name if nc.partition_id_tensor else None

    def run(in_maps):
        per_core = [
            [np.array([[c]], np.uint32) if nm == pid_name else np.asarray(m[nm])
             for nm in in_names]
            for c, m in enumerate(in_maps)]
        concat_in = [
            np.concatenate([per_core[c][i] for c in range(n_cores)], axis=0)
            for i in range(n_params)]
        concat_zeros = [
            np.zeros((n_cores * a.shape[0], *a.shape[1:]), a.dtype)
            for a in out_avals]
        out_arrs = sharded(*concat_in, *concat_zeros)
        out_arrs = [np.asarray(a) for a in out_arrs]
        return [
            {nm: out_arrs[i].reshape(n_cores, *out_avals[i].shape)[c]
             for i, nm in enumerate(out_names)}
            for c in range(n_cores)]

    return run


def kernel(pos, params):
    import os

    pos = np.asarray(pos, F32)

    def _np(tree):
        if isinstance(tree, dict):
            return {k: _np(v) for k, v in tree.items()}
        if isinstance(tree, (list, tuple)):
            return type(tree)(_np(v) for v in tree)
        return np.asarray(tree)

    params = _np(params)

    if "nc" not in _CACHED:
        _CACHED["nc"] = build_kernel()
    nc = _CACHED["nc"]

    wt = prep_params(params)
    in_maps = []
    coords = []
    for b in range(B):
        st, c = prep_sample(pos[b])
        m = dict(wt)
        m.update(st)
        in_maps.append(m)
        coords.append(c)

    if "runner" not in _CACHED:
        _CACHED["runner"] = _make_runner(nc, B)
    results = _CACHED["runner"](in_maps)
    _CACHED["last_in_maps"] = in_maps
    feats = np.stack([np.ascontiguousarray(r["feat_out"].T) for r in results])
    return np.stack(coords), feats


# revision 20
# speedup vs baseline: 23.1431x; 23.1431x over previous
"""PTv3 sparse encoder Trainium2 kernel.

Self-contained: kernel(pos, params) -> (coords [B,1024,3], feat [B,256,1024]).

Strategy: data-parallel over batch B=4 (one sample per NeuronCore, cores 0-3).
Host precomputes Morton-code sort permutations (composed gather index maps),
pooled-coords trajectory (== coords output), and folded weights (LayerNorm
gamma/beta folded into following linear layers, qkv bias augmentation, v-bias
folded into proj bias, attention scale folded into q weights).

Device computes the full feature network per sample:
  token-major activations x[128, T, C]; transposed companions built via
  PE-transpose for matmul inputs; patch attention with transposed scores
  (S^T[m,n]), ACT exp (no max subtraction: softmax is shift-invariant and
  scores are O(1) at this model scale), AV with V as stationary operand
  (col-tiled by head, ones-column producing softmax denominators), deferred
  normalization via SEL/BSEL matmuls + one tensor_tensor multiply.
  Token reorders between blocks via SWDGE dma_gather from a DRAM scratch.
Matmuls run in float32r (4x faster than fp32 on PE at N>=256).
"""
import numpy as np
from contextlib import ExitStack

B, N, GRID = 4, 8192, 0.003
CHS = [32, 64, 128, 256]
DEPTHS = [1, 1, 2, 2]
HEADS = [2, 4, 8, 16]
PATCH = 1024
HD = 16
CPAD = [64, 64, 128, 256]   # padded row size for 256B-multiple gather rows
NS = [8192, 4096, 2048, 1024]
F32 = np.float32

# ------------------------------------------------------------------
# host: morton / orders / coords
# ------------------------------------------------------------------

def _part1by2(x):
    x = x.astype(np.uint32) & np.uint32(0x3FF)
    x = (x | (x << np.uint32(16))) & np.uint32(0x030000FF)
    x = (x | (x << np.uint32(8))) & np.uint32(0x0300F00F)
    x = (x | (x << np.uint32(4))) & np.uint32(0x030C30C3)
    x = (x | (x << np.uint32(2))) & np.uint32(0x09249249)
    return x


def _morton(gc):
    return (_part1by2(gc[..., 0])
            | (_part1by2(gc[..., 1]) << np.uint32(1))
            | (_part1by2(gc[..., 2]) << np.uint32(2)))


def host_orders(pos_b):
    coords = pos_b.astype(F32)
    gmaps, pool_gmaps = [], []
    final_map = None
    for s in range(4):
        n = coords.shape[0]
        gs = GRID * (2 ** s)
        gc = np.clip(np.floor(coords / F32(gs)).astype(np.int32), 0, 1023)
        code_z = _morton(gc)
        code_zt = _morton(gc[..., ::-1])
        ordv = None
        stage_maps = []
        for bi in range(DEPTHS[s]):
            code = code_z if bi % 2 == 0 else code_zt
            perm = np.argsort(code, axis=0, kind="stable").astype(np.int64)
            if ordv is None:
                stage_maps.append(perm)
            else:
                stage_maps.append(np.argsort(ordv, kind="stable")[perm])
            ordv = perm
        gmaps.append(stage_maps)
        if s < 3:
            perm_z = np.argsort(code_z, axis=0, kind="stable").astype(np.int64)
            pm = np.argsort(ordv, kind="stable")[perm_z]
            if np.array_equal(pm, np.arange(n)):
                pm = None
            pool_gmaps.append(pm)
            cs = coords[perm_z]
            coords = cs.reshape(n // 2, 2, 3).mean(axis=1).astype(F32)
        else:
            final_map = np.argsort(ordv, kind="stable")
    return gmaps, pool_gmaps, final_map, coords


def wrap_idx(m):
    """index map -> int16 [128, n/16] wrapped (i -> [i%16, i//16], replicated)."""
    n = m.shape[0]
    t = np.zeros((16, n // 16), dtype=np.int16)
    t[np.arange(n) % 16, np.arange(n) // 16] = m.astype(np.int16)
    return np.tile(t, (8, 1))


# ------------------------------------------------------------------
# host: weight folding / packing
# ------------------------------------------------------------------

def _pack_rows(W):
    """[R, C] -> [128, ceil(R/128), C] zero padded."""
    R, C = W.shape
    nch = (R + 127) // 128
    out = np.zeros((128, nch, C), F32)
    for j in range(nch):
        r = W[j * 128:(j + 1) * 128]
        out[: r.shape[0], j] = r
    return out


def _gelu_tanh(x):
    c = F32(np.sqrt(2.0 / np.pi))
    x = x.astype(F32)
    return (F32(0.5) * x * (F32(1) + np.tanh(c * (x + F32(0.044715) * x ** 3)))).astype(F32)


def embed_row(params):
    e = params['embed']
    v = np.asarray(e['w'], F32)[0] + np.asarray(e['b'], F32)
    m = v.mean(dtype=F32)
    var = v.var(dtype=F32)
    r = F32(1.0) / np.sqrt(var + F32(1e-5))
    return _gelu_tanh((v - m) * r * np.asarray(e['g'], F32) + np.asarray(e['beta'], F32))


def fold_block(p, C, H):
    Hq = (H + 3) // 4
    w = {}
    Wc = np.asarray(p['cpe_w'], F32); bc = np.asarray(p['cpe_b'], F32)
    w['cpe'] = _pack_rows(np.concatenate([Wc, bc[None]], 0))

    g1 = np.asarray(p['ln1_g'], F32); b1 = np.asarray(p['ln1_b'], F32)
    Wqkv = np.asarray(p['qkv_w'], F32); cqkv = np.asarray(p['qkv_b'], F32)
    Wq, Wk, Wv = Wqkv[:, :C], Wqkv[:, C:2 * C], Wqkv[:, 2 * C:]
    cq, ck, cv = cqkv[:C], cqkv[C:2 * C], cqkv[2 * C:]
    sc = F32(0.25)
    Wq_f = np.concatenate([(g1[:, None] * Wq) * sc, ((b1 @ Wq + cq) * sc)[None]], 0)
    Wk_f = np.concatenate([g1[:, None] * Wk, (b1 @ Wk + ck)[None]], 0)
    Wv_f = np.concatenate([g1[:, None] * Wv, np.zeros((1, C), F32)], 0)
    bv_f = b1 @ Wv + cv
    qs = np.zeros((C + 1, Hq * 256), F32)
    for h in range(H):
        g, a = h // 4, h % 4
        qs[:, g * 256 + a * 32: g * 256 + a * 32 + 16] = Wq_f[:, h * 16:(h + 1) * 16]
        qs[:, g * 256 + 128 + a * 32: g * 256 + 128 + a * 32 + 16] = Wk_f[:, h * 16:(h + 1) * 16]
    w['qk'] = _pack_rows(qs)
    w['v'] = _pack_rows(Wv_f)

    Wp = np.asarray(p['proj_w'], F32)
    bp = (np.asarray(p['proj_b'], F32) + bv_f @ Wp).astype(F32)
    ps = np.zeros((Hq * 128, C), F32)
    for h in range(H):
        g, a = h // 4, h % 4
        ps[g * 128 + a * 32: g * 128 + a * 32 + 16] = Wp[h * 16:(h + 1) * 16]
    ps[16] = bp  # rides the s-row (==1 after normalize) of quad 0 head 0
    w['proj'] = ps.reshape(Hq, 128, C).transpose(1, 0, 2).copy()  # [128, Hq, C]

    g2 = np.asarray(p['ln2_g'], F32); b2 = np.asarray(p['ln2_b'], F32)
    W1 = np.asarray(p['fc1_w'], F32); c1 = np.asarray(p['fc1_b'], F32)
    w['fc1'] = _pack_rows(np.concatenate([g2[:, None] * W1, (b2 @ W1 + c1)[None]], 0))
    W2 = np.asarray(p['fc2_w'], F32); c2 = np.asarray(p['fc2_b'], F32)
    w['fc2'] = _pack_rows(np.concatenate([W2, c2[None]], 0))
    return w


def fold_pool(pp, C, Cn):
    W = np.asarray(pp['w'], F32); b = np.asarray(pp['b'], F32)
    return {
        'w': _pack_rows(np.concatenate([W, b[None]], 0)),
        'g': np.asarray(pp['g'], F32)[None, :],
        'beta': np.asarray(pp['beta'], F32)[None, :],
    }


def prep_params(params):
    t = {}
    for s in range(4):
        C, H = CHS[s], HEADS[s]
        for bi in range(DEPTHS[s]):
            w = fold_block(params['stages'][s]['blocks'][bi], C, H)
            for k, v in w.items():
                t[f"w{s}{bi}_{k}"] = np.ascontiguousarray(v)
        if s < 3:
            wp = fold_pool(params['stages'][s]['pool'], C, CHS[s + 1])
            t[f"p{s}_w"] = np.ascontiguousarray(wp['w'])
            t[f"p{s}_g"] = wp['g']
            t[f"p{s}_beta"] = wp['beta']
    x0 = np.zeros((128, 64), F32)
    x0[:, :32] = embed_row(params)[None, :]
    t["x0bc"] = x0
    sel = np.zeros((128, 4), F32)
    for a in range(4):
        sel[32 * a + 16, a] = 1.0
    t["selc"] = sel
    bsel = np.zeros((4, 128), F32)
    for a in range(4):
        bsel[a, 32 * a:32 * a + 32] = 1.0
    t["bselc"] = bsel
    return t


def prep_sample(pos_b):
    gmaps, pool_gmaps, final_map, coords = host_orders(pos_b)
    t = {}
    t["gi_s1"] = wrap_idx(gmaps[1][0])
    t["gi_s2"] = wrap_idx(gmaps[2][0])
    t["gi_s2b"] = wrap_idx(gmaps[2][1])
    t["gi_s2p"] = wrap_idx(pool_gmaps[2])
    t["gi_s3"] = wrap_idx(gmaps[3][0])
    t["gi_s3b"] = wrap_idx(gmaps[3][1])
    t["gi_fin"] = wrap_idx(final_map)
    assert pool_gmaps[0] is None and pool_gmaps[1] is None
    return t, coords


# ------------------------------------------------------------------
# bass kernel
# ------------------------------------------------------------------

_CACHED = {}

WSHAPES = {}
for _s in range(4):
    _C, _H = CHS[_s], HEADS[_s]
    _Hq = (_H + 3) // 4
    _n1 = (_C + 1 + 127) // 128
    WSHAPES[_s] = {
        "cpe": (128, _n1, _C),
        "qk": (128, _n1, _Hq * 256),
        "v": (128, _n1, _C),
        "proj": (128, _Hq, _C),
        "fc1": (128, _n1, 2 * _C),
        "fc2": (128, (2 * _C + 1 + 127) // 128, _C),
    }


def build_kernel():
    import concourse.bacc as bacc
    import concourse.bass as bass
    import concourse.tile as tile
    from concourse import mybir
    from concourse.masks import make_identity

    f32 = mybir.dt.float32
    f32r = mybir.dt.float32r
    i16 = mybir.dt.int16
    AF = mybir.ActivationFunctionType
    OP = mybir.AluOpType

    nc = bacc.Bacc("TRN2", target_bir_lowering=False, debug=False)

    din = {}

    def dI(name, shape, dt=f32):
        din[name] = nc.dram_tensor(name, list(shape), dt, kind="ExternalInput")
        return din[name]

    for s in range(4):
        C = CHS[s]
        for bi in range(DEPTHS[s]):
            for k, shp in WSHAPES[s].items():
                dI(f"w{s}{bi}_{k}", shp)
        if s < 3:
            dI(f"p{s}_w", (128, (C + 1 + 127) // 128, 2 * C))
            dI(f"p{s}_g", (1, 2 * C))
            dI(f"p{s}_beta", (1, 2 * C))
    dI("x0bc", (128, 64))
    dI("selc", (128, 4))
    dI("bselc", (4, 128))
    for nm, n in [("gi_s1", 4096), ("gi_s2", 2048), ("gi_s2b", 2048),
                  ("gi_s2p", 2048), ("gi_s3", 1024), ("gi_s3b", 1024),
                  ("gi_fin", 1024)]:
        dI(nm, (128, n // 16), i16)

    d_out = nc.dram_tensor("feat_out", [1024, 256], f32, kind="ExternalOutput")
    d_scr = nc.dram_tensor("scratch", [8192, 64], f32, kind="Internal")
    d_scr2 = nc.dram_tensor("scratch2", [2048, 128], f32, kind="Internal")
    d_scr3 = nc.dram_tensor("scratch3", [1024, 256], f32, kind="Internal")

    with tile.TileContext(nc) as tc, ExitStack() as ctx:
        ctx.enter_context(nc.allow_low_precision(
            reason="fp32r rounding of matmul inputs (intentional)"))
        sb1 = ctx.enter_context(tc.tile_pool(name="persist", bufs=1))
        wpool = ctx.enter_context(tc.tile_pool(name="wpool", bufs=1))
        spool = ctx.enter_context(tc.tile_pool(name="trans", bufs=2))
        hpool = ctx.enter_context(tc.tile_pool(name="htrans", bufs=1))
        pp_big = ctx.enter_context(tc.tile_pool(name="ppb", bufs=3, space="PSUM"))
        pp_o = ctx.enter_context(tc.tile_pool(name="ppo", bufs=1, space="PSUM"))

        x_flat = sb1.tile([128, 4096], f32, tag="x")
        q_sp = sb1.tile([128, 4, 1024], f32r, tag="qsp")
        k_sp = sb1.tile([128, 4, 1024], f32r, tag="ksp")
        v_buf = sb1.tile([128, 4096], f32, tag="vbuf")
        xTa = sb1.tile([128, 2, 1024], f32r, tag="xTa")
        hTa = sb1.tile([128, 2, 1024], f32r, tag="hTa")
        on_flat = sb1.tile([128, 4096], f32r, tag="on")
        ident = sb1.tile([128, 128], f32, tag="ident")
        ident_r = sb1.tile([128, 128], f32r, tag="identr")
        ones1 = sb1.tile([1, 1024], f32r, tag="ones1")
        eps_t = sb1.tile([128, 1], f32, tag="eps")
        sel_t = sb1.tile([128, 4], f32r, tag="sel")
        bsel_t = sb1.tile([4, 128], f32r, tag="bsel")

        make_identity(nc, ident[:])
        nc.vector.tensor_copy(ident_r[:], ident[:])
        nc.vector.memset(ones1[:].bitcast(f32), 1.0)
        nc.vector.memset(eps_t[:], 1e-5)
        nc.gpsimd.dma_start(sel_t[:], din["selc"][:].bitcast(f32r))
        nc.gpsimd.dma_start(bsel_t[:], din["bselc"][:].bitcast(f32r))

        # pool-stage small weights persistent
        wsb = {}
        for s in range(3):
            C = CHS[s]
            d = din[f"p{s}_w"]
            t = sb1.tile(list(d.shape), f32r, tag=f"p{s}w")
            nc.sync.dma_start(t[:], d[:].bitcast(f32r))
            wsb[f"p{s}_w"] = t
            for nm in ["g", "beta"]:
                dd = din[f"p{s}_{nm}"]
                tt = sb1.tile([128, 2 * C], f32, tag=f"p{s}{nm}")
                src = bass.AP(tensor=dd[:].tensor, offset=dd[:].offset,
                              ap=[[0, 128]] + [list(a) for a in dd[:].ap[1:]])
                nc.gpsimd.dma_start(tt[:], src)
                wsb[f"p{s}_{nm}"] = tt
        gidx = {}
        for nm in ["gi_s1", "gi_s2", "gi_s2b", "gi_s2p", "gi_s3", "gi_s3b", "gi_fin"]:
            t = sb1.tile(list(din[nm].shape), i16, tag=nm)
            nc.sync.dma_start(t[:], din[nm][:])
            gidx[nm] = t

        gsem = nc.alloc_semaphore("gsem")
        sem_count = [0]

        def x_view(s):
            T = NS[s] // 128
            return x_flat[:, 0:T * CPAD[s]].rearrange("p (t c) -> p t c", c=CPAD[s])

        def dram_gather(s_from, s_to, idx_t, n_to):
            Cp = CPAD[s_from]
            scr = {64: d_scr, 128: d_scr2, 256: d_scr3}[Cp]
            xv = x_view(s_from)
            T_from = NS[s_from] // 128
            dview = scr[:].rearrange("(t p) c -> p t c", p=128)
            xo = x_view(s_to)
            with tc.tile_critical():
                sem_count[0] += 16
                nc.gpsimd.dma_start(
                    dview[:, 0:T_from, 0:Cp], xv[:, :, :]).then_inc(gsem, 16)
                nc.gpsimd.wait_ge(gsem, sem_count[0])
                sem_count[0] += 16
                nc.gpsimd.dma_gather(
                    out_ap=xo[:, 0:n_to // 128, :],
                    in_ap=scr[0:NS[s_from], :],
                    idxs_ap=idx_t[:],
                    num_idxs=n_to,
                    num_idxs_reg=n_to,
                    elem_size=Cp,
                    single_packet=False,
                ).then_inc(gsem, 16)
                nc.gpsimd.wait_ge(gsem, sem_count[0])

        def transpose_in(src_ap_fn, ncols, dst_chunk, n_tok):
            """PE-transpose n_tok//128 token-major tiles into dst rows."""
            ntile = n_tok // 128
            for grp in range(0, ntile, 4):
                gn = min(4, ntile - grp)
                pt = pp_big.tile([128, 4, 128], f32, tag="big")
                for t in range(gn):
                    nc.tensor.transpose(
                        pt[0:ncols, t, :], src_ap_fn(grp + t), ident[:])
                nc.vector.tensor_copy(
                    dst_chunk[0:ncols, grp * 128:(grp + gn) * 128],
                    pt[0:ncols, 0:gn, :])

        def load_w(s, bi):
            lw = {}
            for k, shp in WSHAPES[s].items():
                t = wpool.tile(list(shp), f32r, tag="w_" + k)
                nc.sync.dma_start(t[:], din[f"w{s}{bi}_{k}"][:].bitcast(f32r))
                lw[k] = t
            return lw

        def ln_apply(xv_ap_fn, out_t, C, gn):
            """LayerNorm (no affine) of gn token tiles -> out_t[:, t, 0:C]."""
            stats = spool.tile([128, 8, 6], f32, tag="st")
            mv = spool.tile([128, 8, 2], f32, tag="mv")
            rs = spool.tile([128, 8], f32, tag="rs")
            for t in range(gn):
                nc.vector.bn_stats(stats[:, t, :], xv_ap_fn(t))
                nc.vector.bn_aggr(mv[:, t, :], stats[:, t, :])
            nc.scalar.activation(rs[:, 0:gn], mv[:, 0:gn, 1], AF.Sqrt,
                                 bias=eps_t[:])
            nc.vector.reciprocal(rs[:, 0:gn], rs[:, 0:gn])
            for t in range(gn):
                rbc = bass.AP(tensor=rs[:].tensor, offset=rs[:].offset + t,
                              ap=[list(rs[:].ap[0]), [0, C]])
                nc.vector.scalar_tensor_tensor(
                    out=out_t[:, t, 0:C], in0=xv_ap_fn(t),
                    scalar=mv[:, t, 0:1], in1=rbc,
                    op0=OP.subtract, op1=OP.mult)

        # ================= block =================
        def run_block(s, bi):
            C, H = CHS[s], HEADS[s]
            n = NS[s]
            P = n // PATCH
            Hq = (H + 3) // 4
            F = 2 * C
            xv = x_view(s)
            lw = load_w(s, bi)
            inline_ones = C < 128
            nch = 1 if inline_ones else C // 128
            kkx = C + 1 if inline_ones else 128
            jr, rr = C // 128, C % 128          # bias row position (rr==0 unless inline)
            fc_inline = (F + 1) <= 128
            nF = (F + 127) // 128
            fjr, frr = F // 128, F % 128
            vv = v_buf[:, 0:8 * H * 32].rearrange("p (t h e) -> p t h e", t=8, h=H)
            o_n = on_flat[:, :].rearrange("p (g c) -> p g c", c=1024)
            gT = o_n

            for p in range(P):
                pt0 = p * 8

                if inline_ones:
                    nc.vector.memset(xTa[C:C + 1, 0, 0:1024].bitcast(f32), 1.0)
                for j in range(nch):
                    cw = min(128, C - j * 128)
                    transpose_in(
                        lambda t, j=j, cw=cw: xv[:, pt0 + t, j * 128:j * 128 + cw],
                        cw, xTa[:, j, :], PATCH)

                # cpe + residual
                for gt in range(0, 8, 4):
                    pc = pp_big.tile([128, 4, 256], f32, tag="big")
                    for t in range(4):
                        tok = (gt + t) * 128
                        for j in range(nch):
                            nc.tensor.matmul(
                                pc[:, t, 0:C], xTa[0:kkx, j, tok:tok + 128],
                                lw["cpe"][0:kkx, j, 0:C],
                                start=(j == 0), stop=(j == nch - 1 and inline_ones))
                        if not inline_ones:
                            nc.tensor.matmul(
                                pc[:, t, 0:C], ones1[:, 0:128],
                                lw["cpe"][rr:rr + 1, jr, 0:C],
                                start=False, stop=True)
                    for t in range(4):
                        nc.vector.tensor_tensor(
                            out=xv[:, pt0 + gt + t, 0:C], in0=pc[:, t, 0:C],
                            in1=xv[:, pt0 + gt + t, 0:C], op=OP.add)

                # ln1 -> h -> hT
                h_t = hpool.tile([128, 8, 256], f32, tag="h")
                ln_apply(lambda t: xv[:, pt0 + t, 0:C], h_t, C, 8)
                if inline_ones:
                    nc.vector.memset(hTa[C:C + 1, 0, 0:1024].bitcast(f32), 1.0)
                for j in range(nch):
                    cw = min(128, C - j * 128)
                    transpose_in(
                        lambda t, j=j, cw=cw: h_t[:, t, j * 128:j * 128 + cw],
                        cw, hTa[:, j, :], PATCH)

                # qkT spread
                for g in range(Hq):
                    for jk, dst in ((0, q_sp), (1, k_sp)):
                        pq = pp_big.tile([128, 1024], f32, tag="big")
                        colb = g * 256 + jk * 128
                        for nh in range(2):
                            for j in range(nch):
                                nc.tensor.matmul(
                                    pq[:, nh * 512:(nh + 1) * 512],
                                    lw["qk"][0:kkx, j, colb:colb + 128],
                                    hTa[0:kkx, j, nh * 512:(nh + 1) * 512],
                                    start=(j == 0),
                                    stop=(j == nch - 1 and inline_ones))
                            if not inline_ones:
                                nc.tensor.matmul(
                                    pq[:, nh * 512:(nh + 1) * 512],
                                    lw["qk"][rr:rr + 1, jr, colb:colb + 128],
                                    ones1[:, nh * 512:(nh + 1) * 512],
                                    start=False, stop=True)
                        nc.vector.tensor_copy(dst[:, g, :], pq[:])

                # V token-major (no bias)
                for gt in range(0, 8, 4):
                    pv = pp_big.tile([128, 4, 256], f32, tag="big")
                    for t in range(4):
                        tok = (gt + t) * 128
                        for j in range(nch):
                            kk = C if inline_ones else 128
                            nc.tensor.matmul(
                                pv[:, t, 0:H * 16],
                                hTa[0:kk, j, tok:tok + 128],
                                lw["v"][0:kk, j, 0:H * 16],
                                start=(j == 0), stop=(j == nch - 1))
                    src = pv[:, 0:4, 0:H * 16].rearrange("p t (h e) -> p t h e", e=16)
                    nc.vector.tensor_copy(vv[:, gt:gt + 4, :, 0:16], src)

                # attention
                for g in range(Hq):
                    hg = min(4, H - 4 * g)
                    R = hg * 32
                    po = pp_o.tile([128, 1024], f32, tag="opsum")
                    for nh in range(2):
                        for mt in range(8):
                            for d2 in range(0, hg, 2):
                                dn = min(2, hg - d2)
                                psc = pp_big.tile([128, 2, 512], f32, tag="big")
                                for a2 in range(dn):
                                    a = d2 + a2
                                    nc.tensor.matmul(
                                        psc[:, a2, :],
                                        k_sp[32 * a:32 * a + 16, g,
                                             mt * 128:(mt + 1) * 128],
                                        q_sp[32 * a:32 * a + 16, g,
                                             nh * 512:(nh + 1) * 512],
                                        start=True, stop=True,
                                        tile_position=(32 * a, 0))
                                E = spool.tile([128, 2, 512], f32, tag="E")
                                nc.scalar.activation(
                                    E[:, 0:dn, :], psc[:, 0:dn, :], AF.Exp)
                                for a2 in range(dn):
                                    a = d2 + a2
                                    nc.tensor.matmul(
                                        po[32 * a:32 * a + 32,
                                           nh * 512:(nh + 1) * 512],
                                        vv[:, mt, 4 * g + a, 0:32],
                                        E[:, a2, :],
                                        start=(mt == 0), stop=(mt == 7),
                                        tile_position=(0, 32 * a),
                                        skip_group_check=True)
                    o_s = spool.tile([128, 1024], f32r, tag="os")
                    nc.vector.tensor_copy(o_s[0:R, :], po[0:R, :])
                    ps4 = pp_big.tile([128, 1024], f32, tag="big")
                    for nh in range(2):
                        nc.tensor.matmul(
                            ps4[0:hg, nh * 512:(nh + 1) * 512],
                            sel_t[0:R, 0:hg],
                            o_s[0:R, nh * 512:(nh + 1) * 512],
                            start=True, stop=True)
                    invs = spool.tile([4, 1024], f32r, tag="invs")
                    nc.vector.reciprocal(invs[0:hg, :], ps4[0:hg, 0:1024])
                    pB = pp_big.tile([128, 1024], f32, tag="big")
                    for nh in range(2):
                        nc.tensor.matmul(
                            pB[0:R, nh * 512:(nh + 1) * 512],
                            bsel_t[0:hg, 0:R],
                            invs[0:hg, nh * 512:(nh + 1) * 512],
                            start=True, stop=True)
                    nc.vector.tensor_tensor(
                        out=o_n[0:R, g, :], in0=o_s[0:R, :], in1=pB[0:R, :],
                        op=OP.mult)

                # proj + residual
                for gt in range(0, 8, 4):
                    pc = pp_big.tile([128, 4, 256], f32, tag="big")
                    for t in range(4):
                        tok = (gt + t) * 128
                        for g in range(Hq):
                            R = min(4, H - 4 * g) * 32
                            nc.tensor.matmul(
                                pc[:, t, 0:C],
                                o_n[0:R, g, tok:tok + 128],
                                lw["proj"][0:R, g, 0:C],
                                start=(g == 0), stop=(g == Hq - 1))
                    for t in range(4):
                        nc.vector.tensor_tensor(
                            out=xv[:, pt0 + gt + t, 0:C], in0=pc[:, t, 0:C],
                            in1=xv[:, pt0 + gt + t, 0:C], op=OP.add)

                # ln2 -> h -> hT
                h2 = hpool.tile([128, 8, 256], f32, tag="h")
                ln_apply(lambda t: xv[:, pt0 + t, 0:C], h2, C, 8)
                if inline_ones:
                    nc.vector.memset(hTa[C:C + 1, 0, 0:1024].bitcast(f32), 1.0)
                for j in range(nch):
                    cw = min(128, C - j * 128)
                    transpose_in(
                        lambda t, j=j, cw=cw: h2[:, t, j * 128:j * 128 + cw],
                        cw, hTa[:, j, :], PATCH)

                # fc1 -> gelu (gT = on_flat chunks)
                for mj in range(nF):
                    fm = min(128, F - mj * 128)
                    pf = pp_big.tile([128, 1024], f32, tag="big")
                    for nh in range(2):
                        for j in range(nch):
                            nc.tensor.matmul(
                                pf[0:fm, nh * 512:(nh + 1) * 512],
                                lw["fc1"][0:kkx, j, mj * 128:mj * 128 + fm],
                                hTa[0:kkx, j, nh * 512:(nh + 1) * 512],
                                start=(j == 0), stop=(j == nch - 1 and inline_ones))
                        if not inline_ones:
                            nc.tensor.matmul(
                                pf[0:fm, nh * 512:(nh + 1) * 512],
                                lw["fc1"][rr:rr + 1, jr, mj * 128:mj * 128 + fm],
                                ones1[:, nh * 512:(nh + 1) * 512],
                                start=False, stop=True)
                    nc.scalar.activation(gT[0:fm, mj, :], pf[0:fm, :],
                                         AF.Gelu_apprx_tanh)
                if fc_inline:
                    nc.vector.memset(gT[F:F + 1, 0, 0:1024].bitcast(f32), 1.0)

                # fc2 + residual
                for gt in range(0, 8, 4):
                    pc2 = pp_big.tile([128, 4, 256], f32, tag="big")
                    for t in range(4):
                        tok = (gt + t) * 128
                        for mj in range(nF):
                            kk = F + 1 if fc_inline else 128
                            nc.tensor.matmul(
                                pc2[:, t, 0:C],
                                gT[0:kk, mj, tok:tok + 128],
                                lw["fc2"][0:kk, mj, 0:C],
                                start=(mj == 0), stop=(mj == nF - 1 and fc_inline))
                        if not fc_inline:
                            nc.tensor.matmul(
                                pc2[:, t, 0:C], ones1[:, 0:128],
                                lw["fc2"][frr:frr + 1, fjr, 0:C],
                                start=False, stop=True)
                    for t in range(4):
                        nc.vector.tensor_tensor(
                            out=xv[:, pt0 + gt + t, 0:C], in0=pc2[:, t, 0:C],
                            in1=xv[:, pt0 + gt + t, 0:C], op=OP.add)

        # ================= pool =================
        def run_pool(s):
            C = CHS[s]
            Cn = 2 * C
            n = NS[s]
            n2 = n // 2
            xv = x_view(s)
            inline_ones = C < 128
            nch = 1 if inline_ones else C // 128
            kkx = C + 1 if inline_ones else 128
            jr, rr = C // 128, C % 128
            nM = (Cn + 127) // 128
            ymT = on_flat[:, 0:nM * n2].rearrange("p (m c) -> p m c", m=nM)
            for pc in range(n // 1024):
                if inline_ones:
                    nc.vector.memset(xTa[C:C + 1, 0, 0:1024].bitcast(f32), 1.0)
                for j in range(nch):
                    cw = min(128, C - j * 128)
                    transpose_in(
                        lambda t, j=j, cw=cw: xv[:, pc * 8 + t, j * 128:j * 128 + cw],
                        cw, xTa[:, j, :], PATCH)
                for mj in range(nM):
                    fm = min(128, Cn - mj * 128)
                    py = pp_big.tile([128, 1024], f32, tag="big")
                    for nh in range(2):
                        for j in range(nch):
                            nc.tensor.matmul(
                                py[0:fm, nh * 512:(nh + 1) * 512],
                                wsb[f"p{s}_w"][0:kkx, j, mj * 128:mj * 128 + fm],
                                xTa[0:kkx, j, nh * 512:(nh + 1) * 512],
                                start=(j == 0), stop=(j == nch - 1 and inline_ones))
                        if not inline_ones:
                            nc.tensor.matmul(
                                py[0:fm, nh * 512:(nh + 1) * 512],
                                wsb[f"p{s}_w"][rr:rr + 1, jr, mj * 128:mj * 128 + fm],
                                ones1[:, nh * 512:(nh + 1) * 512],
                                start=False, stop=True)
                    ptmp = spool.tile([128, 512], f32, tag="pm")
                    nc.vector.tensor_copy(ptmp[0:fm, :], py[0:fm, 0:1024:2])
                    nc.vector.tensor_tensor(
                        out=ymT[0:fm, mj, pc * 512:(pc + 1) * 512],
                        in0=ptmp[0:fm, :],
                        in1=py[0:fm, 1:1024:2], op=OP.max)
            # transpose back token-major into next-stage x view
            xo = x_view(s + 1)
            T2 = n2 // 128
            for grp in range(0, T2, 4):
                gn = min(4, T2 - grp)
                for mj in range(nM):
                    ncols = min(128, Cn - mj * 128)
                    ptb = pp_big.tile([128, 4, 128], f32r, tag="big")
                    for t in range(gn):
                        nc.tensor.transpose(
                            ptb[0:128, t, 0:ncols],
                            ymT[0:ncols, mj, (grp + t) * 128:(grp + t + 1) * 128],
                            ident_r[0:ncols, 0:ncols])
                    nc.vector.tensor_copy(
                        xo[:, grp:grp + gn, mj * 128:mj * 128 + ncols],
                        ptb[:, 0:gn, 0:ncols])
            # ln * g + beta, gelu
            Gt = wsb[f"p{s}_g"]
            Bt = wsb[f"p{s}_beta"]
            for grp in range(0, T2, 8):
                gn = min(8, T2 - grp)
                h_t = hpool.tile([128, 8, 256], f32, tag="h")
                ln_apply(lambda t: xo[:, grp + t, 0:Cn], h_t, Cn, gn)
                gbc = bass.AP(tensor=Gt[:].tensor, offset=Gt[:].offset,
                              ap=[list(Gt[:].ap[0]), [0, gn], [1, Cn]])
                bbc = bass.AP(tensor=Bt[:].tensor, offset=Bt[:].offset,
                              ap=[list(Bt[:].ap[0]), [0, gn], [1, Cn]])
                nc.vector.tensor_tensor(out=h_t[:, 0:gn, 0:Cn],
                                        in0=h_t[:, 0:gn, 0:Cn], in1=gbc, op=OP.mult)
                nc.vector.tensor_tensor(out=h_t[:, 0:gn, 0:Cn],
                                        in0=h_t[:, 0:gn, 0:Cn], in1=bbc, op=OP.add)
                nc.scalar.activation(xo[:, grp:grp + gn, 0:Cn],
                                     h_t[:, 0:gn, 0:Cn], AF.Gelu_apprx_tanh)

        def stage_init(s):
            H = HEADS[s]
            vv = v_buf[:, 0:8 * H * 32].rearrange("p (t h e) -> p t h e", t=8, h=H)
            nc.vector.memset(vv[:, :, :, 16:17], 1.0)
            nc.vector.memset(vv[:, :, :, 17:32], 0.0)

        # ================= main =================
        x0v = x_view(0)
        src = bass.AP(tensor=din["x0bc"][:].tensor, offset=din["x0bc"][:].offset,
                      ap=[[64, 128], [0, 64], [1, 64]])
        nc.gpsimd.dma_start(x0v[:, :, :], src)

        import os as _os
        _nph = int(_os.environ.get("KBENCH_PHASES", "99"))
        phases = [
            lambda: (stage_init(0), run_block(0, 0)),
            lambda: run_pool(0),
            lambda: dram_gather(1, 1, gidx["gi_s1"], 4096),
            lambda: (stage_init(1), run_block(1, 0)),
            lambda: run_pool(1),
            lambda: dram_gather(2, 2, gidx["gi_s2"], 2048),
            lambda: (stage_init(2), run_block(2, 0)),
            lambda: dram_gather(2, 2, gidx["gi_s2b"], 2048),
            lambda: run_block(2, 1),
            lambda: dram_gather(2, 2, gidx["gi_s2p"], 2048),
            lambda: run_pool(2),
            lambda: dram_gather(3, 3, gidx["gi_s3"], 1024),
            lambda: (stage_init(3), run_block(3, 0)),
            lambda: dram_gather(3, 3, gidx["gi_s3b"], 1024),
            lambda: run_block(3, 1),
            lambda: dram_gather(3, 3, gidx["gi_fin"], 1024),
        ]
        for _f in phases[:_nph]:
            _f()
        if _nph >= len(phases):
            xf = x_view(3)
            nc.sync.dma_start(
                d_out[:].rearrange("(t p) c -> p t c", p=128), xf[:, :, :])
        else:
            nc.vector.memset(x_flat[0:1, 0:1], 0.0)
            nc.sync.dma_start(
                d_out[:].rearrange("(t p) c -> p t c", p=128),
                x_flat[:, 0:2048].rearrange("p (t c) -> p t c", c=256))

    nc.finalize()
    return nc


def _make_runner(nc, n_cores):
    """Cached PJRT runner (mirrors bass2jax.run_bass_via_pjrt but reusable)."""
    import jax
    import numpy as _np
    from jax.sharding import Mesh, PartitionSpec
    from jax.experimental.shard_map import shard_map
    import concourse.mybir as mybir
    from concourse.bass2jax import install_neuronx_cc_hook, _bass_exec_p

    install_neuronx_cc_hook()
    in_names, out_names, out_avals = [], [], []
    for alloc in nc.m.functions[0].allocations:
        if not isinstance(alloc, mybir.MemoryLocationSet):
            continue
        name = alloc.memorylocations[0].name
        if alloc.kind == "ExternalInput":
            in_names.append(name)
        elif alloc.kind == "ExternalOutput":
            shape = tuple(alloc.tensor_shape)
            dtype = mybir.dt.np(alloc.dtype)
            out_names.append(name)
            out_avals.append(jax.core.ShapedArray(shape, dtype))
    n_params = len(in_names)
    n_outs = len(out_avals)
    all_names = in_names + out_names

    def _body(*args):
        outs = _bass_exec_p.bind(
            *args,
            out_avals=tuple(out_avals),
            in_names=tuple(all_names),
            out_names=tuple(out_names),
            lowering_input_output_aliases=(),
            sim_require_finite=True,
            sim_require_nnan=True,
            nc=nc,
        )
        return tuple(outs)

    devices = jax.devices()[:n_cores]
    mesh = Mesh(_np.asarray(devices), ("core",))
    in_specs = (PartitionSpec("core"),) * (n_params + n_outs)
    out_specs = (PartitionSpec("core"),) * n_outs
    donate = tuple(range(n_params, n_params + n_outs))
    sharded = jax.jit(
        shard_map(_body, mesh=mesh, in_specs=in_specs, out_specs=out_specs,
                  check_rep=False),
        donate_argnums=donate, keep_unused=True)

    pid_name = nc.partition_id_tensor.name if nc.partition_id_tensor else None

    def run(in_maps):
        per_core = [
            [np.array([[c]], np.uint32) if nm == pid_name else np.asarray(m[nm])
             for nm in in_names]
            for c, m in enumerate(in_maps)]
        concat_in = [
            np.concatenate([per_core[c][i] for c in range(n_cores)], axis=0)
            for i in range(n_params)]
        concat_zeros = [
            np.zeros((n_cores * a.shape[0], *a.shape[1:]), a.dtype)
            for a in out_avals]
        out_arrs = sharded(*concat_in, *concat_zeros)
        out_arrs = [np.asarray(a) for a in out_arrs]
        return [
            {nm: out_arrs[i].reshape(n_cores, *out_avals[i].shape)[c]
             for i, nm in enumerate(out_names)}
            for c in range(n_cores)]

    return run


def kernel(pos, params):
    import os

    pos = np.asarray(pos, F32)

    def _np(tree):
        if isinstance(tree, dict):
            return {k: _np(v) for k, v in tree.items()}
        if isinstance(tree, (list, tuple)):
            return type(tree)(_np(v) for v in tree)
        return np.asarray(tree)

    params = _np(params)

    if "nc" not in _CACHED:
        _CACHED["nc"] = build_kernel()
    nc = _CACHED["nc"]

    wt = prep_params(params)
    in_maps = []
    coords = []
    for b in range(B):
        st, c = prep_sample(pos[b])
        m = dict(wt)
        m.update(st)
        in_maps.append(m)
        coords.append(c)

    if "runner" not in _CACHED:
        _CACHED["runner"] = _make_runner(nc, B)
    results = _CACHED["runner"](in_maps)
    _CACHED["last_in_maps"] = in_maps
    feats = np.stack([np.ascontiguousarray(r["feat_out"].T) for r in results])
    return np.stack(coords), feats
